# revision 6
# baseline (speedup 1.0000x reference)
"""APPNP (gcn_norm + K-step propagation) on 8 Trainium2 NeuronCores — v3.

v3 changes vs v2:
  - Dense slot packing: edges packed contiguously per (chunk, class) run
    (sorted by window within the run), no per-window 128-padding.  Blocks
    (128-slot matmul units) may span window boundaries; each (block,
    window) pair present in ANY core gets its own 64-column one-hot S
    slice and its own accumulating matmul.  Cuts gather descriptors from
    ~258k to ~204k per step (the gather is descriptor-rate-bound).
  - (from v2) Compact 512B-row gather table via 5 source-ptile classes;
    AllGather payload 1.31MB/core; single int16 row space.
  - Pad slots point at spread-out rows (same-row descriptors serialize).
"""

import hashlib

import numpy as np
import ml_dtypes

import jax
import jax.numpy as jnp
from jax.sharding import Mesh, PartitionSpec, NamedSharding
from jax.experimental.shard_map import shard_map

import concourse.bass as bass
import concourse.bacc as bacc
import concourse.tile as tile
import concourse.mybir as mybir
from concourse import ap_utils
from concourse import bass2jax
from concourse.bass import MemorySpace

# ---------------- problem constants ----------------
N = 100000
E = 1600000
D = 48
K = 5
ALPHA = 0.8
NCORES = 8
SHARD = 12500
SHARD_PAD = 12544            # 98 * 128
NT = SHARD_PAD // 128        # 98 ptiles
W = 64                       # dst nodes per window
NWIN = SHARD_PAD // W        # 196 windows
WIN_PER_CHUNK = 20           # windows per chunk (= 10 ptiles, 480 psum cols)
NCHUNK = (NWIN + WIN_PER_CHUNK - 1) // WIN_PER_CHUNK  # 10
CLS = 5                      # source classes (ptile bands)
CLS_PT = 20                  # ptiles per class (last class has 18)
ROWS_PER_CORE = CLS_PT * 128  # 2560
TROWS = NCORES * ROWS_PER_CORE  # 20480 table rows
TSTRIDE = 256                # bf16 elems per table row (512B)
MAX_GATHER = 16128           # idxs per dma_gather instruction (ring limit)
NQUEUES = 4                  # SWDGE queues for gather parallelism
SAFE_ROW = 2304              # rows [0, SAFE_ROW) of core 0 are always written

bf16 = mybir.dt.bfloat16
f32 = mybir.dt.float32
fp8 = mybir.dt.float8e4
i16 = mybir.dt.int16

FP8_ONE = 0x38  # 1.0 in e4m3


def cls_of_ptile(t):
    return min(t // CLS_PT, CLS - 1)


def cls_ptiles(cl):
    t0 = cl * CLS_PT
    return t0, min(NT - t0, CLS_PT)


def raw_dma_gather(gp, out_ap, in_ap, idxs_ap, num_idxs, elem_size, elem_step,
                   queue_num=0):
    """bass.dma_gather without the elem_size%256 assert (non-transpose HBM)."""
    assert idxs_ap.dtype == mybir.dt.int16
    assert in_ap.dtype == out_ap.dtype
    assert in_ap.space == MemorySpace.DRAM
    assert idxs_ap.space == MemorySpace.SBUF
    assert out_ap.space == MemorySpace.SBUF
    assert ap_utils.ap_is_contiguous(in_ap.ap[1:])
    assert ap_utils.ap_is_contiguous(out_ap.ap[1:])
    assert ap_utils.ap_is_contiguous(idxs_ap.ap[1:])
    assert in_ap.ap[-1][1] == out_ap.ap[-1][1] == elem_size
    assert out_ap.ap[0][1] * out_ap.ap[1][1] == num_idxs
    assert in_ap.ap[0][0] == elem_step
    stride_bytes = elem_step * mybir.dt.size(in_ap.dtype)
    stride_bytes_256 = stride_bytes // 256
    assert stride_bytes_256 * 256 == stride_bytes and stride_bytes_256 < 256
    _in_ap = gp.lower_ap_dma(in_ap, for_custom_bir_dma=True)
    _idxs_ap = gp.lower_ap(idxs_ap)
    _out_ap = gp.lower_ap(out_ap)
    return gp.add_instruction(
        mybir.InstDMAGatherAnt(
            name=gp.bass.get_next_instruction_name(),
            ins=[*_in_ap, _idxs_ap, gp.lower_val_access(gp.to_reg(num_idxs))],
            outs=[_out_ap],
            transpose=False,
            num_idxs=num_idxs,
            elem_size=elem_size,
            stride_bytes_256=stride_bytes_256,
            gen_mode=0,
            single_packet=(num_idxs <= 1024),
            queue_num=queue_num,
            sbuf_tokens_per_rank=0,
            sbuf_free_dim_per_rank=0,
            sbuf_free_dim_pad_per_rank=0,
            sbuf_byte_offset=0,
        )
    )


# ---------------- host-side structure preprocessing ----------------

def preprocess(edge_index):
    """Build the uniform (shared across cores) block structure + per-core data.

    Slots are packed densely per (chunk, class) run: each core's edges of the
    run sit at run_start + rank (sorted by window), the run is padded to the
    max count over cores rounded to 128.  Matmul structure is the union over
    cores of (block, window) incidences.
    """
    src = np.asarray(edge_index[0], dtype=np.int64)
    dst = np.asarray(edge_index[1], dtype=np.int64)
    # table row + class of each source node
    score = src // SHARD
    j = src % SHARD
    t = j // 128
    scls_all = np.minimum(t // CLS_PT, CLS - 1)
    srow_all = score * ROWS_PER_CORE + (j - scls_all * ROWS_PER_CORE)

    NRUN = NCHUNK * CLS

    def wins_in_chunk(ch):
        return min(WIN_PER_CHUNK, NWIN - ch * WIN_PER_CHUNK)

    # per core: edges sorted by (run, window, original order)
    cores = []
    counts = np.zeros((NCORES, NRUN), dtype=np.int64)          # edges per run
    # window-boundary cumulative counts per (core, run, window 0..20)
    wcum = np.zeros((NCORES, NRUN, WIN_PER_CHUNK + 1), dtype=np.int64)
    for c in range(NCORES):
        lo, hi = c * SHARD, (c + 1) * SHARD
        sel = (dst >= lo) & (dst < hi)
        ld = dst[sel] - lo
        srow = srow_all[sel]
        scl = scls_all[sel]
        w = ld >> 6                       # global window in shard
        ch = w // WIN_PER_CHUNK
        wl = w % WIN_PER_CHUNK
        run = ch * CLS + scl
        # Interleave edges across HBM bank classes (srow low bits) within
        # each (run, window) segment: same-bank gather descriptors spaced
        # apart cycle banks like a sequential sweep (~20% faster gathers).
        seg = run * WIN_PER_CHUNK + wl
        bank4 = (srow >> 1) & 15
        so = np.lexsort((bank4, seg))          # segment-major, bank-grouped
        seg_s, bank_s = seg[so], bank4[so]
        segstart = np.zeros(len(so), dtype=np.int64)
        newseg = np.ones(len(so), dtype=bool)
        newseg[1:] = seg_s[1:] != seg_s[:-1]
        newbank = np.ones(len(so), dtype=bool)
        newbank[1:] = newseg[1:] | (bank_s[1:] != bank_s[:-1])
        # rank within (segment, bank) bucket
        idxs = np.arange(len(so))
        bstart = np.maximum.accumulate(np.where(newbank, idxs, 0))
        rank_in_bucket = idxs - bstart
        # Decorrelate bucket phase across runs/segments: concurrent queues
        # cycling buckets in lockstep collide on the same HBM bank.
        a_seg = (((seg_s * 2654435761) >> 8) & 7) * 2 + 1
        b_seg = (seg_s * 11) & 15
        key = rank_in_bucket * 16 + ((bank_s * a_seg + b_seg) & 15)
        order = so[np.lexsort((key, seg_s))]
        run_s = run[order]
        wl_s = wl[order]
        counts[c] = np.bincount(run_s, minlength=NRUN)
        for r in range(NRUN):
            m = run_s == r
            wcum[c, r, 1:] = np.cumsum(
                np.bincount(wl_s[m], minlength=WIN_PER_CHUNK))
        cores.append((ld[order], srow[order], run_s))

    run_len = ((counts.max(axis=0) + 127) // 128) * 128
    run_start = np.zeros(NRUN, dtype=np.int64)
    run_start[1:] = np.cumsum(run_len)[:-1]
    nslots = int(run_len.sum())
    nblk = nslots // 128

    # (block, window) pairs: union over cores.  Pairs ordered by
    # (chunk, class, block, window).
    pairs = []          # (blk_global, win_global)
    chunk_pair_first = np.zeros(NCHUNK + 1, dtype=np.int64)
    chunk_first = np.zeros(NCHUNK + 1, dtype=np.int64)
    runs = []           # (block_lo, block_hi, cls)
    for ch in range(NCHUNK):
        chunk_first[ch] = run_start[ch * CLS] // 128
        chunk_pair_first[ch] = len(pairs)
        for cl in range(CLS):
            r = ch * CLS + cl
            b0 = run_start[r] // 128
            nb = run_len[r] // 128
            if nb:
                runs.append((b0, b0 + nb, cl))
            wmin = wcum[:, r, :-1].min(axis=0)   # min start per window
            wmax = wcum[:, r, 1:].max(axis=0)    # max end per window
            for b in range(nb):
                bs, be = b * 128, (b + 1) * 128
                for wl in range(wins_in_chunk(ch)):
                    if wmin[wl] < be and wmax[wl] > bs:
                        pairs.append((b0 + b, ch * WIN_PER_CHUNK + wl))
    chunk_first[NCHUNK] = nblk
    chunk_pair_first[NCHUNK] = len(pairs)
    npairs = len(pairs)

    struct = dict(
        nslots=nslots, nblk=nblk, npairs=npairs, pairs=pairs,
        chunk_first=chunk_first, chunk_pair_first=chunk_pair_first,
        runs=runs,
    )

    # pair lookup: (blk, win) -> pair index
    pair_idx = {}
    for p, (b, wv) in enumerate(pairs):
        pair_idx[(b, wv)] = p

    per_core = []
    for c in range(NCORES):
        ld_s, srow_s, run_s = cores[c]
        cnt = counts[c]
        cstart = np.zeros(NRUN, dtype=np.int64)
        cstart[1:] = np.cumsum(cnt)[:-1]
        rank = np.arange(len(ld_s)) - np.repeat(cstart, cnt)
        slot = np.repeat(run_start, cnt) + rank
        # Pad slots must NOT all point at one row: same-row gather descriptors
        # serialize in HBM.  Spread them over always-written rows of core 0.
        idx_all = (np.arange(nslots) % SAFE_ROW).astype(np.int16)
        idx_all[slot] = srow_s.astype(np.int16)
        # one-hot S with multiplicity: slot s, window wv -> pair (s//128, wv)
        blk_e = slot // 128
        win_e = (ld_s >> 6)
        pcol = np.fromiter(
            (pair_idx[(b, wv)] for b, wv in zip(blk_e.tolist(), win_e.tolist())),
            dtype=np.int64, count=len(blk_e))
        s_f = np.zeros((128, npairs * W), dtype=np.float32)
        np.add.at(s_f, (slot % 128, pcol * W + (ld_s & 63)), 1.0)
        idxw = np.tile(idx_all.reshape(-1, 16).T, (8, 1)).copy()  # [128, ns/16]
        deg = np.ones((128, NT, 1), dtype=np.float32)
        lo = c * SHARD
        bc = np.bincount(dst[(dst >= lo) & (dst < lo + SHARD)] - lo,
                         minlength=SHARD_PAD).astype(np.float32)
        deg[:, :, 0] += bc.reshape(NT, 128).T
        per_core.append(dict(idx=idxw, s=s_f.astype(ml_dtypes.float8_e4m3),
                             deg=deg))
    return struct, per_core


# ---------------- device program ----------------

def build_program(struct, nsteps):
    nslots = struct["nslots"]
    nblk = struct["nblk"]
    npairs = struct["npairs"]
    pairs = struct["pairs"]
    chunk_first = struct["chunk_first"]
    chunk_pair_first = struct["chunk_pair_first"]
    runs = struct["runs"]

    nc = bacc.Bacc("TRN2", target_bir_lowering=False, debug=False,
                   num_devices=NCORES, num_swdge_queues=NQUEUES)

    x_in = nc.declare_dram_parameter("x_shard", [SHARD_PAD, D], f32, isOutput=False)
    deg_in = nc.declare_dram_parameter("deg", [128, NT, 1], f32, isOutput=False)
    idx_in = nc.declare_dram_parameter("idx", [128, nslots // 16], i16, isOutput=False)
    s_in = nc.declare_dram_parameter("s_mat", [128, npairs * W], fp8, isOutput=False)
    out_ext = nc.declare_dram_parameter("out", [SHARD_PAD, D], f32, isOutput=True)

    g_local = nc.dram_tensor("g_local", [ROWS_PER_CORE, TSTRIDE], bf16)
    g_full = nc.dram_tensor("g_full", [TROWS, TSTRIDE], bf16, addr_space="Shared")

    cc_sem = nc.alloc_semaphore("cc_sem")
    qn = [0]  # round-robin gather queue counter

    with tile.TileContext(nc) as tc:
        with (
            tc.tile_pool(name="perm", bufs=1) as perm,
            tc.tile_pool(name="work", bufs=2) as work,
            tc.tile_pool(name="psum", bufs=4, space="PSUM") as psum_pool,
        ):
            # ---- init: x, h, xa, dinv ----
            h_t = perm.tile([128, NT, D], f32)
            xa_t = perm.tile([128, NT, D], f32)
            nc.sync.dma_start(out=xa_t[:],
                              in_=x_in[:].rearrange("(t p) d -> p t d", p=128))
            nc.vector.tensor_copy(h_t[:], xa_t[:])
            nc.vector.tensor_scalar_mul(xa_t[:], xa_t[:], ALPHA)

            deg_t = work.tile([128, NT, 1], f32, tag="deg")
            nc.sync.dma_start(out=deg_t[:], in_=deg_in[:])
            dinv_t = perm.tile([128, NT, 1], f32)
            nc.scalar.activation(out=dinv_t[:], in_=deg_t[:],
                                 func=mybir.ActivationFunctionType.Sqrt)
            nc.vector.reciprocal(dinv_t[:], dinv_t[:])
            dinv02_t = perm.tile([128, NT, 1], f32)
            nc.vector.tensor_scalar_mul(dinv02_t[:], dinv_t[:], 1.0 - ALPHA)

            idx_t = perm.tile([128, nslots // 16], i16)
            nc.sync.dma_start(out=idx_t[:], in_=idx_in[:])

            gbf_t = perm.tile([128, NT, D], bf16)

            # g_local viewed as [128, CLS_PT, TSTRIDE]; class cl's features
            # live at cols [cl*D, (cl+1)*D) of its ptile-band rows.
            gl_r = g_local[:].rearrange("(t p) s -> p t s", p=128)

            if nsteps > 0:
                nc.vector.tensor_tensor(
                    out=gbf_t[:], in0=h_t[:],
                    in1=dinv_t[:].to_broadcast([128, NT, D]),
                    op=mybir.AluOpType.mult)
                for cl in range(CLS):
                    t0, tn = cls_ptiles(cl)
                    nc.sync.dma_start(
                        out=gl_r[:, 0:tn, cl * D : (cl + 1) * D],
                        in_=gbf_t[:, t0 : t0 + tn, :])

            for step in range(nsteps):
                with tc.tile_critical():
                    nc.gpsimd.collective_compute(
                        "AllGather",
                        mybir.AluOpType.bypass,
                        ins=[g_local[:]],
                        outs=[g_full[:]],
                        replica_groups=[list(range(NCORES))],
                    ).then_inc(cc_sem, 1)
                    nc.gpsimd.wait_ge(cc_sem, step + 1)

                for ch in range(NCHUNK):
                    b0, b1 = int(chunk_first[ch]), int(chunk_first[ch + 1])
                    p0, p1 = int(chunk_pair_first[ch]), int(chunk_pair_first[ch + 1])
                    nb = b1 - b0
                    msgs = work.tile([128, nb, D], bf16, tag="msgs")
                    s_t = work.tile([128, (p1 - p0) * W], fp8, tag="s")
                    nc.sync.dma_start(out=s_t[:], in_=s_in[:, p0 * W : p1 * W])
                    # gathers: per (chunk, class) run, split to <= MAX_GATHER
                    for (rlo, rhi, cl) in runs:
                        if rlo < b0 or rlo >= b1:
                            continue
                        table = g_full[0:, cl * D : (cl + 1) * D]
                        pos = rlo
                        while pos < rhi:
                            pe = min(rhi, pos + MAX_GATHER // 128)
                            n_idx = (pe - pos) * 128
                            raw_dma_gather(
                                nc.gpsimd,
                                msgs[:, pos - b0 : pe - b0, :],
                                table,
                                idx_t[:, pos * 8 : pe * 8],
                                n_idx, D, TSTRIDE,
                                queue_num=qn[0] % NQUEUES,
                            )
                            qn[0] += 1
                            pos = pe
                    ps = psum_pool.tile([128, (WIN_PER_CHUNK // 2) * D], f32, tag="ps")
                    first_wp, last_wp = {}, {}
                    for p in range(p0, p1):
                        wp = (pairs[p][1] - ch * WIN_PER_CHUNK) % 2
                        if wp not in first_wp:
                            first_wp[wp] = p
                        last_wp[wp] = p
                    for p in range(p0, p1):
                        b, wv = pairs[p]
                        wl = wv - ch * WIN_PER_CHUNK
                        wp = wl % 2
                        tl = wl // 2
                        nc.tensor.matmul(
                            out=ps[wp * 64 : (wp + 1) * 64, tl * D : (tl + 1) * D],
                            lhsT=s_t[:, (p - p0) * W : (p - p0 + 1) * W],
                            rhs=msgs[:, b - b0, :],
                            start=(p == first_wp[wp]),
                            stop=(p == last_wp[wp]),
                            skip_group_check=True,
                        )
                    # h update for this chunk's ptiles
                    t0 = ch * (WIN_PER_CHUNK // 2)
                    tn = min(NT - t0, WIN_PER_CHUNK // 2)
                    sl = slice(t0, t0 + tn)
                    ncol = tn * D
                    tmp1 = work.tile([128, (WIN_PER_CHUNK // 2) * D], f32, tag="tmp1")
                    nc.vector.tensor_tensor(
                        out=tmp1[:, :ncol].rearrange("p (t d) -> p t d", d=D),
                        in0=h_t[:, sl, :],
                        in1=dinv_t[:, sl, :].to_broadcast([128, tn, D]),
                        op=mybir.AluOpType.mult)
                    nc.vector.tensor_tensor(
                        out=tmp1[:, :ncol], in0=tmp1[:, :ncol],
                        in1=ps[:, :ncol], op=mybir.AluOpType.add)
                    nc.vector.tensor_tensor(
                        out=tmp1[:, :ncol].rearrange("p (t d) -> p t d", d=D),
                        in0=tmp1[:, :ncol].rearrange("p (t d) -> p t d", d=D),
                        in1=dinv02_t[:, sl, :].to_broadcast([128, tn, D]),
                        op=mybir.AluOpType.mult)
                    nc.vector.tensor_tensor(
                        out=h_t[:, sl, :],
                        in0=tmp1[:, :ncol].rearrange("p (t d) -> p t d", d=D),
                        in1=xa_t[:, sl, :],
                        op=mybir.AluOpType.add)
                    if step < nsteps - 1:
                        # refresh this chunk's slice of the gather table.
                        # chunk ch lies entirely within class ch//2.
                        cl = ch // 2
                        ct0, _ = cls_ptiles(cl)
                        lt0 = t0 - ct0
                        nc.vector.tensor_tensor(
                            out=gbf_t[:, sl, :], in0=h_t[:, sl, :],
                            in1=dinv_t[:, sl, :].to_broadcast([128, tn, D]),
                            op=mybir.AluOpType.mult)
                        nc.sync.dma_start(
                            out=gl_r[:, lt0 : lt0 + tn, cl * D : (cl + 1) * D],
                            in_=gbf_t[:, sl, :])

            nc.sync.dma_start(
                out=out_ext[:].rearrange("(t p) d -> p t d", p=128), in_=h_t[:])

    nc.compile()
    return nc


# ---------------- persistent PJRT executable ----------------

class PersistentRunner:
    """Compile a Bass program once; run it many times with device-resident
    inputs."""

    def __init__(self, nc, n_cores):
        bass2jax.install_neuronx_cc_hook()
        self.nc = nc
        self.n_cores = n_cores
        in_names, out_names, out_avals = [], [], []
        partition_name = nc.partition_id_tensor.name if nc.partition_id_tensor else None
        for alloc in nc.m.functions[0].allocations:
            if not isinstance(alloc, mybir.MemoryLocationSet):
                continue
            name = alloc.memorylocations[0].name
            if alloc.kind == "ExternalInput":
                if name != partition_name:
                    in_names.append(name)
            elif alloc.kind == "ExternalOutput":
                out_names.append(name)
                out_avals.append(
                    jax.core.ShapedArray(tuple(alloc.tensor_shape),
                                         mybir.dt.np(alloc.dtype)))
        self.in_names = in_names
        self.out_names = out_names
        self.out_avals = out_avals
        n_params = len(in_names)
        n_outs = len(out_avals)
        all_names = in_names + out_names
        if partition_name is not None:
            all_names = all_names + [partition_name]

        def _body(*args):
            operands = list(args)
            if partition_name is not None:
                operands.append(bass2jax.partition_id_tensor())
            outs = bass2jax._bass_exec_p.bind(
                *operands,
                out_avals=tuple(out_avals),
                in_names=tuple(all_names),
                out_names=tuple(out_names),
                lowering_input_output_aliases=(),
                sim_require_finite=True,
                sim_require_nnan=True,
                nc=nc,
            )
            return tuple(outs)

        devices = jax.devices()[:n_cores]
        self.mesh = Mesh(np.asarray(devices), ("core",))
        in_specs = (PartitionSpec("core"),) * (n_params + n_outs)
        out_specs = (PartitionSpec("core"),) * n_outs
        donate = tuple(range(n_params, n_params + n_outs))
        self.sharded = jax.jit(
            shard_map(_body, mesh=self.mesh, in_specs=in_specs,
                      out_specs=out_specs, check_rep=False),
            donate_argnums=donate,
            keep_unused=True,
        )
        self.spec = NamedSharding(self.mesh, PartitionSpec("core"))

    def put_inputs(self, in_maps):
        concat = [
            np.concatenate([np.asarray(in_maps[c][n])
                            for c in range(self.n_cores)], axis=0)
            for n in self.in_names
        ]
        return [jax.device_put(a, self.spec) for a in concat]

    def zeros(self):
        return [
            jax.device_put(
                jnp.zeros((self.n_cores * av.shape[0], *av.shape[1:]), av.dtype),
                self.spec)
            for av in self.out_avals
        ]

    def run_blocked(self, dev_inputs):
        outs = self.sharded(*dev_inputs, *self.zeros())
        for o in outs:
            o.block_until_ready()
        return outs


# ---------------- kernel entry point ----------------

_PROGRAMS = {}
_DEV_INPUTS = {}


def _get_program(edge_index, nsteps):
    ekey = (hashlib.sha1(np.ascontiguousarray(edge_index)).hexdigest(), nsteps)
    if ekey not in _PROGRAMS:
        struct, per_core = preprocess(edge_index)
        nc = build_program(struct, nsteps)
        runner = PersistentRunner(nc, NCORES)
        _PROGRAMS[ekey] = (runner, struct, per_core)
    return ekey, _PROGRAMS[ekey]


def _get_dev_inputs(ekey, runner, per_core, x):
    xkey = ekey + (hashlib.sha1(np.ascontiguousarray(x)).hexdigest(),)
    if xkey not in _DEV_INPUTS:
        in_maps = []
        for c in range(NCORES):
            xs = np.zeros((SHARD_PAD, D), dtype=np.float32)
            xs[:SHARD] = x[c * SHARD : (c + 1) * SHARD]
            in_maps.append({
                "x_shard": xs,
                "deg": per_core[c]["deg"],
                "idx": per_core[c]["idx"],
                "s_mat": per_core[c]["s"],
            })
        _DEV_INPUTS[xkey] = runner.put_inputs(in_maps)
    return _DEV_INPUTS[xkey]


def _run(inputs, nsteps=K):
    x = np.asarray(inputs["x"], dtype=np.float32)
    edge_index = np.asarray(inputs["edge_index"])
    ekey, (runner, struct, per_core) = _get_program(edge_index, nsteps)
    dev_in = _get_dev_inputs(ekey, runner, per_core, x)
    outs = runner.run_blocked(dev_in)
    res = np.asarray(outs[0]).reshape(NCORES, SHARD_PAD, D)
    out = np.empty((N, D), dtype=np.float32)
    for c in range(NCORES):
        out[c * SHARD : (c + 1) * SHARD] = res[c][:SHARD]
    return out


def kernel(**inputs):
    return _run(inputs, nsteps=K)
